# revision 3
# baseline (speedup 1.0000x reference)
"""Attention-pooling kernel (AttLayer) for Trainium2, data-parallel over batch
across 8 NeuronCores.

  uit = tanh(x @ W + b)            [B, T, A]
  ait = exp(uit @ u) * mask        [B, T]
  out = einsum('btd,bt->bd', x, ait / (sum_t ait + eps))

Shapes are hardcoded: x [64, 4096, 256] f32, W [256, 32], b [32], u [32, 1],
mask [64, 4096] bool. Each core handles 8 batches; per batch the sequence is
processed in 32 chunks of [128 t x 256 d].

Per-chunk dataflow: DMA x tile (f32, 128KB contiguous) -> DVE convert to bf16
-> PE transpose (matmul by identity, bf16) -> ACT copy PSUM->SBUF -> two PE
matmuls accumulate x@W into a per-batch PSUM region [128, 32chunks*32a].
Per batch: DVE adds bias (+ additive mask bias), ACT tanh, DVE mul by u,
DVE reduce -> scores [128, 32]; ACT exp -> e (bf16); 32 PE matmuls
(e_chunk^T @ x_bf, N=256) accumulate the numerator [1, 256]; one matmul
computes the denominator; reciprocal+scale; DMA out.
"""

import os
import sys

sys.path.insert(0, "/opt/trn_rl_repo")

import numpy as np

import concourse.bass as bass
import concourse.mybir as mybir
import concourse.tile as tile
from concourse import bacc
from concourse.bass import ds, ts
from concourse.bass_utils import run_bass_kernel_spmd

F32 = mybir.dt.float32
BF16 = mybir.dt.bfloat16

N_CORES = 8
B, T, D, A = 64, 4096, 256, 32
BPC = B // N_CORES          # batches per core
NCH = T // 128              # 128-row chunks per batch
EPS = 1e-7
MASK_BIAS = 30.0            # additive pre-exp mask: s + (mask-1)*30

last_exec_time_ns = None


def _build():
    nc = bacc.Bacc(None, target_bir_lowering=False, debug=True)

    x_dram = nc.dram_tensor("x", [BPC, T, D], F32, kind="ExternalInput")
    w_dram = nc.dram_tensor("w", [128, 2 * A], F32, kind="ExternalInput")
    bbt_dram = nc.dram_tensor("bbt", [128, NCH * A], F32, kind="ExternalInput")
    ubt_dram = nc.dram_tensor("ubt", [128, NCH * A], F32, kind="ExternalInput")
    maskb_dram = nc.dram_tensor("maskb", [BPC, 128, NCH], F32, kind="ExternalInput")
    ident_dram = nc.dram_tensor("ident", [128, 128], F32, kind="ExternalInput")
    out_dram = nc.dram_tensor("out", [BPC, D], F32, kind="ExternalOutput")

    with tile.TileContext(nc) as tc:
        with (
            tc.tile_pool(name="const", bufs=1) as cpool,
            tc.tile_pool(name="xf", bufs=6) as xfpool,
            tc.tile_pool(name="xbf", bufs=2 * NCH) as xbfpool,
            tc.tile_pool(name="xt", bufs=3) as xtpool,
            tc.tile_pool(name="ph2", bufs=2) as ph2pool,
            tc.tile_pool(name="small", bufs=2) as spool,
            tc.tile_pool(name="uitps", bufs=2, space="PSUM") as uitpool,
            tc.tile_pool(name="xtps", bufs=2, space="PSUM") as xtpspool,
            tc.tile_pool(name="ops", bufs=1, space="PSUM") as opool,
            tc.tile_pool(name="denps", bufs=1, space="PSUM") as denpool,
        ):
            # ---- constants (one-time) ----
            w_f32 = cpool.tile([128, 2 * A], F32, name="w_f32")
            nc.sync.dma_start(out=w_f32[:], in_=w_dram[:])
            w_bf = cpool.tile([128, 2 * A], BF16, name="w_bf")
            nc.vector.tensor_copy(w_bf[:], w_f32[:])

            ident_f32 = cpool.tile([128, 128], F32, name="ident_f32")
            nc.sync.dma_start(out=ident_f32[:], in_=ident_dram[:])
            ident_bf = cpool.tile([128, 128], BF16, name="ident_bf")
            nc.vector.tensor_copy(ident_bf[:], ident_f32[:])

            bbt = cpool.tile([128, NCH * A], F32, name="bbt")
            nc.sync.dma_start(out=bbt[:], in_=bbt_dram[:])
            ubt = cpool.tile([128, NCH * A], F32, name="ubt")
            nc.sync.dma_start(out=ubt[:], in_=ubt_dram[:])

            ones_bf = cpool.tile([128, 1], BF16, name="ones_bf")
            nc.vector.memset(ones_bf[:], 1.0)

            for bb in range(BPC):
                uit_ps = uitpool.tile([128, NCH * A], F32, name="uit_ps", tag="uit")
                x_bf_tiles = []
                for i in range(NCH):
                    x_f32 = xfpool.tile([128, D], F32, name="x_f32", tag="xf")
                    nc.sync.dma_start(out=x_f32[:], in_=x_dram[bb][ts(i, 128), :])
                    x_bf = xbfpool.tile([128, D], BF16, name="x_bf", tag="xbf")
                    nc.vector.tensor_copy(x_bf[:], x_f32[:])
                    x_bf_tiles.append(x_bf)

                    xt_ps = xtpspool.tile([128, D], BF16, name="xt_ps", tag="xtps")
                    nc.tensor.transpose(xt_ps[:, 0:128], x_bf[:, 0:128], ident_bf[:])
                    nc.tensor.transpose(xt_ps[:, 128:256], x_bf[:, 128:256], ident_bf[:])
                    xt_sb = xtpool.tile([128, D], BF16, name="xt_sb", tag="xt")
                    nc.scalar.copy(xt_sb[:], xt_ps[:])

                    nc.tensor.matmul(
                        uit_ps[:, ds(A * i, A)],
                        lhsT=xt_sb[:, 0:128],
                        rhs=w_bf[:, 0:A],
                        start=True,
                        stop=False,
                    )
                    nc.tensor.matmul(
                        uit_ps[:, ds(A * i, A)],
                        lhsT=xt_sb[:, 128:256],
                        rhs=w_bf[:, A : 2 * A],
                        start=False,
                        stop=True,
                    )

                # ---- phase 2: scores for the whole batch ----
                t1 = ph2pool.tile([128, NCH * A], F32, name="t1", tag="t1")
                nc.vector.tensor_add(t1[:], uit_ps[:], bbt[:])
                t2 = ph2pool.tile([128, NCH * A], F32, name="t2", tag="t2")
                nc.scalar.activation(t2[:], t1[:], mybir.ActivationFunctionType.Tanh)
                t3 = ph2pool.tile([128, NCH * A], F32, name="t3", tag="t3")
                nc.vector.tensor_mul(t3[:], t2[:], ubt[:])
                s_all = spool.tile([128, NCH, 1], F32, name="s_all", tag="s_all")
                nc.vector.reduce_sum(
                    s_all[:],
                    t3.rearrange("p (i a) -> p i a", a=A),
                    axis=mybir.AxisListType.X,
                )

                maskb = spool.tile([128, NCH], F32, name="maskb", tag="maskb")
                nc.sync.dma_start(out=maskb[:], in_=maskb_dram[bb])
                s_m = spool.tile([128, NCH], F32, name="s_m", tag="s_m")
                nc.vector.tensor_add(s_m[:], s_all[:, :, 0], maskb[:])

                e_bf = spool.tile([128, NCH], BF16, name="e_bf", tag="e_bf")
                nc.scalar.activation(e_bf[:], s_m[:], mybir.ActivationFunctionType.Exp)

                er = spool.tile([128, 1], F32, name="er", tag="er")
                nc.vector.reduce_sum(er[:], e_bf[:], axis=mybir.AxisListType.X)
                erb = spool.tile([128, 1], BF16, name="erb", tag="erb")
                nc.vector.tensor_copy(erb[:], er[:])

                den_ps = denpool.tile([1, 1], F32, name="den_ps", tag="den")
                nc.tensor.matmul(
                    den_ps[:], lhsT=erb[:], rhs=ones_bf[:], start=True, stop=True
                )

                # ---- phase 3: weighted sum over the sequence ----
                o_ps = opool.tile([1, D], F32, name="o_ps", tag="o")
                for i in range(NCH):
                    nc.tensor.matmul(
                        o_ps[:],
                        lhsT=e_bf[:, ds(i, 1)],
                        rhs=x_bf_tiles[i][:],
                        start=(i == 0),
                        stop=(i == NCH - 1),
                    )

                # ---- phase 4: finalize ----
                den_sb = spool.tile([1, 1], F32, name="den_sb", tag="den_sb")
                nc.vector.tensor_scalar_add(den_sb[:], den_ps[:], EPS)
                inv = spool.tile([1, 1], F32, name="inv", tag="inv")
                nc.vector.reciprocal(inv[:], den_sb[:])
                o_sb = spool.tile([1, D], F32, name="o_sb", tag="o_sb")
                nc.vector.tensor_scalar_mul(o_sb[:], o_ps[:], inv[:])
                nc.sync.dma_start(out=out_dram[bb][None, :], in_=o_sb[:])

    nc.finalize()
    return nc


def kernel(x, mask, W, b, u):
    global last_exec_time_ns
    x = np.ascontiguousarray(np.asarray(x), dtype=np.float32)
    mask_f = np.asarray(mask).astype(np.float32)
    W = np.asarray(W, dtype=np.float32)
    b = np.asarray(b, dtype=np.float32)
    u = np.asarray(u, dtype=np.float32)

    # host-side layout prep (all tiny except x, which is only view-sliced)
    w_packed = np.ascontiguousarray(
        W.reshape(2, 128, A).transpose(1, 0, 2).reshape(128, 2 * A)
    )
    bbt = np.ascontiguousarray(np.tile(b[None, :], (128, NCH)))
    ubt = np.ascontiguousarray(np.tile(u[:, 0][None, :], (128, NCH)))
    # mask -> additive pre-exp bias, laid out [b][p][chunk] with t = 128*chunk + p
    maskb = np.ascontiguousarray(
        ((mask_f - 1.0) * MASK_BIAS).reshape(B, NCH, 128).transpose(0, 2, 1)
    )
    ident = np.eye(128, dtype=np.float32)

    nc = _build()

    in_maps = []
    for c in range(N_CORES):
        in_maps.append(
            {
                "x": x[c * BPC : (c + 1) * BPC],
                "w": w_packed,
                "bbt": bbt,
                "ubt": ubt,
                "maskb": maskb[c * BPC : (c + 1) * BPC],
                "ident": ident,
            }
        )

    trace = bool(int(os.environ.get("BASS_KERNEL_TRACE", "0")))
    res = run_bass_kernel_spmd(
        nc, in_maps, core_ids=list(range(N_CORES)), trace=trace
    )
    last_exec_time_ns = res.exec_time_ns

    out = np.empty((B, D), dtype=np.float32)
    for c in range(N_CORES):
        out[c * BPC : (c + 1) * BPC] = res.results[c]["out"]
    return out


# revision 6
# speedup vs baseline: 1.2308x; 1.2308x over previous
"""Attention-pooling kernel (AttLayer) for Trainium2, data-parallel over batch
across 8 NeuronCores.

  uit = tanh(x @ W + b)            [B, T, A]
  ait = exp(uit @ u) * mask        [B, T]
  out = einsum('btd,bt->bd', x, ait / (sum_t ait + eps))

Shapes hardcoded: x [64, 4096, 256] f32, W [256, 32], b [32], u [32, 1],
mask [64, 4096] bool. Each core handles 8 batches.

Layout: per batch, T=4096 rows are loaded in 2 contiguous 2MB DMAs of
[128, 16, 256] (partition p holds rows 16p..16p+15 of its half), i.e.
t = 2048 g + 16 p + r.  A "chunk" i = 16 g + r is a [128 t x 256 d] slab
whose within-chunk position is the partition index p.

Per chunk: PE transposes the two [128, 128] d-blocks (float32r, matmul by
identity) into PSUM; DVE/ACT copy them to SBUF as bf16 (alternating full
[128, 512] tiles to amortize fixed costs); two bf16 PE matmuls accumulate
x@W into a per-batch PSUM region [128, 32*32].  Per batch: DVE adds bias
(+ additive mask bias), ACT tanh, DVE mul by u and reduce -> scores
[128, 32]; ACT exp -> e; 32 PE matmuls (e_i^T @ x_i, float32r, N=256)
accumulate the numerator [1, 256]; one matmul forms the denominator;
reciprocal + scale; DMA out.
"""

import os
import sys

sys.path.insert(0, "/opt/trn_rl_repo")

import numpy as np

import concourse.bass as bass
import concourse.mybir as mybir
import concourse.tile as tile
from concourse import bacc
from concourse.bass import ds, ts
from concourse.bass_utils import run_bass_kernel_spmd

F32 = mybir.dt.float32
F32R = mybir.dt.float32r
BF16 = mybir.dt.bfloat16

N_CORES = 8
B, T, D, A = 64, 4096, 256, 32
BPC = B // N_CORES          # batches per core
NCH = T // 128              # 128-row chunks per batch (32)
NG = 2                      # DMA groups per batch
RPG = NCH // NG             # chunks per group (16)
EPS = 1e-7
MASK_BIAS = 30.0            # additive pre-exp mask: s + (mask-1)*30

last_exec_time_ns = None


def _build():
    nc = bacc.Bacc(None, target_bir_lowering=False, debug=True)

    x_dram = nc.dram_tensor("x", [BPC, T, D], F32R, kind="ExternalInput")
    w_dram = nc.dram_tensor("w", [128, 2 * A], F32, kind="ExternalInput")
    bbt_dram = nc.dram_tensor("bbt", [128, NCH * A], F32, kind="ExternalInput")
    ubt_dram = nc.dram_tensor("ubt", [128, NCH * A], F32, kind="ExternalInput")
    maskb_dram = nc.dram_tensor("maskb", [BPC, 128, NCH], F32, kind="ExternalInput")
    ident_dram = nc.dram_tensor("ident", [128, 128], F32, kind="ExternalInput")
    out_dram = nc.dram_tensor("out", [BPC, D], F32, kind="ExternalOutput")

    with tile.TileContext(nc) as tc:
        with (
            tc.tile_pool(name="const", bufs=1) as cpool,
            tc.tile_pool(name="xf", bufs=4) as xfpool,
            tc.tile_pool(name="xt", bufs=4) as xtpool,
            tc.tile_pool(name="ph2", bufs=2) as ph2pool,
            tc.tile_pool(name="small", bufs=2) as spool,
            tc.tile_pool(name="uitps", bufs=2, space="PSUM") as uitpool,
            tc.tile_pool(name="xtps", bufs=2, space="PSUM") as xtpspool,
            tc.tile_pool(name="ops", bufs=1, space="PSUM") as opool,
            tc.tile_pool(name="denps", bufs=1, space="PSUM") as denpool,
        ):
            # ---- constants (one-time) ----
            w_f32 = cpool.tile([128, 2 * A], F32, name="w_f32")
            nc.sync.dma_start(out=w_f32[:], in_=w_dram[:])
            w_bf = cpool.tile([128, 2 * A], BF16, name="w_bf")
            nc.vector.tensor_copy(w_bf[:], w_f32[:])

            ident = cpool.tile([128, 128], F32, name="ident")
            nc.sync.dma_start(out=ident[:], in_=ident_dram[:])
            ident_r = cpool.tile([128, 128], F32R, name="ident_r")
            nc.vector.tensor_copy(ident_r[:], ident[:])

            bbt = cpool.tile([128, NCH * A], F32, name="bbt")
            nc.sync.dma_start(out=bbt[:], in_=bbt_dram[:])
            ubt = cpool.tile([128, NCH * A], F32, name="ubt")
            nc.sync.dma_start(out=ubt[:], in_=ubt_dram[:])

            ones_f = cpool.tile([128, 1], F32, name="ones_f")
            nc.vector.memset(ones_f[:], 1.0)

            for bb in range(BPC):
                uit_ps = uitpool.tile([128, NCH * A], F32, name="uit_ps", tag="uit")
                x_grp_tiles = []
                for g in range(NG):
                    x_grp = xfpool.tile([128, RPG, D], F32R, name="x_grp", tag="xf")
                    nc.sync.dma_start(
                        out=x_grp[:],
                        in_=x_dram[bb][ds(2048 * g, 2048), :].rearrange(
                            "(p r) d -> p r d", r=RPG
                        ),
                    )
                    x_grp_tiles.append(x_grp)

                    # transpose chunks in pairs; copy PSUM->SBUF one [128,512]
                    # tile at a time, alternating DVE/ACT
                    for rp in range(RPG // 2):
                        xt_ps = xtpspool.tile([128, 2, D], F32R, name="xt_ps", tag="xtps")
                        for rr in range(2):
                            r = 2 * rp + rr
                            for dc in range(2):
                                nc.tensor.transpose(
                                    xt_ps[:, rr, ds(128 * dc, 128)],
                                    x_grp[:, r, ds(128 * dc, 128)],
                                    ident_r[:],
                                )
                        xt_sb = xtpool.tile([128, 2, D], BF16, name="xt_sb", tag="xt")
                        if rp % 2 == 0:
                            nc.vector.tensor_copy(xt_sb[:], xt_ps[:].bitcast(F32))
                        else:
                            nc.scalar.copy(xt_sb[:], xt_ps[:].bitcast(F32))
                        for rr in range(2):
                            i = 16 * g + 2 * rp + rr
                            nc.tensor.matmul(
                                uit_ps[:, ds(A * i, A)],
                                lhsT=xt_sb[:, rr, 0:128],
                                rhs=w_bf[:, 0:A],
                                start=True,
                                stop=False,
                            )
                            nc.tensor.matmul(
                                uit_ps[:, ds(A * i, A)],
                                lhsT=xt_sb[:, rr, 128:256],
                                rhs=w_bf[:, A : 2 * A],
                                start=False,
                                stop=True,
                            )

                # ---- phase 2: scores for the whole batch ----
                t1 = ph2pool.tile([128, NCH * A], F32, name="t1", tag="t1")
                nc.vector.tensor_add(t1[:], uit_ps[:], bbt[:])
                t2 = ph2pool.tile([128, NCH * A], F32, name="t2", tag="t2")
                nc.scalar.activation(t2[:], t1[:], mybir.ActivationFunctionType.Tanh)
                t3 = ph2pool.tile([128, NCH * A], F32, name="t3", tag="t3")
                nc.vector.tensor_mul(t3[:], t2[:], ubt[:])
                s_all = spool.tile([128, NCH, 1], F32, name="s_all", tag="s_all")
                nc.vector.reduce_sum(
                    s_all[:],
                    t3.rearrange("p (i a) -> p i a", a=A),
                    axis=mybir.AxisListType.X,
                )

                maskb = spool.tile([128, NCH], F32, name="maskb", tag="maskb")
                nc.sync.dma_start(out=maskb[:], in_=maskb_dram[bb])
                s_m = spool.tile([128, NCH], F32, name="s_m", tag="s_m")
                nc.vector.tensor_add(s_m[:], s_all[:, :, 0], maskb[:])

                e_f = spool.tile([128, NCH], F32, name="e_f", tag="e_f")
                nc.scalar.activation(e_f[:], s_m[:], mybir.ActivationFunctionType.Exp)

                e_r = spool.tile([128, NCH], F32R, name="e_r", tag="e_r")
                nc.vector.tensor_copy(e_r[:], e_f[:])
                er = spool.tile([128, 1], F32, name="er", tag="er")
                nc.vector.reduce_sum(er[:], e_f[:], axis=mybir.AxisListType.X)

                den_ps = denpool.tile([1, 1], F32, name="den_ps", tag="den")
                nc.tensor.matmul(
                    den_ps[:],
                    lhsT=er[:],
                    rhs=ones_f[:],
                    start=True,
                    stop=True,
                )

                # ---- phase 3: weighted sum over the sequence ----
                o_ps = opool.tile([1, D], F32, name="o_ps", tag="o")
                for i in range(NCH):
                    g, r = divmod(i, RPG)
                    nc.tensor.matmul(
                        o_ps[:],
                        lhsT=e_r[:, ds(i, 1)],
                        rhs=x_grp_tiles[g][:, r, :],
                        start=(i == 0),
                        stop=(i == NCH - 1),
                    )

                # ---- phase 4: finalize ----
                den_sb = spool.tile([1, 1], F32, name="den_sb", tag="den_sb")
                nc.vector.tensor_scalar_add(den_sb[:], den_ps[:], EPS)
                inv = spool.tile([1, 1], F32, name="inv", tag="inv")
                nc.vector.reciprocal(inv[:], den_sb[:])
                o_sb = spool.tile([1, D], F32, name="o_sb", tag="o_sb")
                nc.vector.tensor_scalar_mul(o_sb[:], o_ps[:], inv[:])
                nc.sync.dma_start(out=out_dram[bb][None, :], in_=o_sb[:])

    nc.finalize()
    return nc


def kernel(x, mask, W, b, u):
    global last_exec_time_ns
    x = np.ascontiguousarray(np.asarray(x), dtype=np.float32)
    mask_f = np.asarray(mask).astype(np.float32)
    W = np.asarray(W, dtype=np.float32)
    b = np.asarray(b, dtype=np.float32)
    u = np.asarray(u, dtype=np.float32)

    # host-side layout prep (all tiny except x, which is only view-sliced)
    w_packed = np.ascontiguousarray(
        W.reshape(2, 128, A).transpose(1, 0, 2).reshape(128, 2 * A)
    )
    bbt = np.ascontiguousarray(np.tile(b[None, :], (128, NCH)))
    ubt = np.ascontiguousarray(np.tile(u[:, 0][None, :], (128, NCH)))
    # mask -> additive pre-exp bias, laid out [b][p][(g r)] with t = 2048g+16p+r
    maskb = np.ascontiguousarray(
        ((mask_f - 1.0) * MASK_BIAS)
        .reshape(B, NG, 128, RPG)
        .transpose(0, 2, 1, 3)
        .reshape(B, 128, NCH)
    )
    ident = np.eye(128, dtype=np.float32)

    nc = _build()

    in_maps = []
    for c in range(N_CORES):
        in_maps.append(
            {
                "x": x[c * BPC : (c + 1) * BPC],
                "w": w_packed,
                "bbt": bbt,
                "ubt": ubt,
                "maskb": maskb[c * BPC : (c + 1) * BPC],
                "ident": ident,
            }
        )

    trace = bool(int(os.environ.get("BASS_KERNEL_TRACE", "0")))
    res = run_bass_kernel_spmd(
        nc, in_maps, core_ids=list(range(N_CORES)), trace=trace
    )
    last_exec_time_ns = res.exec_time_ns

    out = np.empty((B, D), dtype=np.float32)
    for c in range(N_CORES):
        out[c * BPC : (c + 1) * BPC] = res.results[c]["out"]
    return out


# revision 7
# speedup vs baseline: 1.3706x; 1.1136x over previous
"""Attention-pooling kernel (AttLayer) for Trainium2, data-parallel over batch
across 8 NeuronCores.

  uit = tanh(x @ W + b)            [B, T, A]
  ait = exp(uit @ u) * mask        [B, T]
  out = einsum('btd,bt->bd', x, ait / (sum_t ait + eps))

Shapes hardcoded: x [64, 4096, 256] f32, W [256, 32], b [32], u [32, 1],
mask [64, 4096] bool. Each core handles 8 batches.

Layout: per batch, T=4096 rows arrive in 2 contiguous 2MB DMAs of
[128, 16, 256] (partition p holds rows 16p..16p+15 of its half), i.e.
t = 2048 g + 16 p + r.  A "chunk" i = 16 g + r is a [128 t x 256 d] slab
whose within-chunk position is the partition index p.

Per group: one DVE copy converts the whole [128, 16*256] slab f32->bf16.
Per chunk: PE transposes the two [128, 128] d-blocks (bf16 matmul by
identity) into PSUM; DVE/ACT copy them back to SBUF (alternating [128, 512]
tiles to amortize fixed costs); two bf16 PE matmuls accumulate x@W into a
per-batch PSUM region [128, 32*32].  Per batch: DVE adds bias, ACT tanh,
DVE mul by u + reduce -> scores [128, 32]; DVE adds the additive mask bias;
ACT exp -> e (bf16) with fused row-sum accum; 32 bf16 PE matmuls
(e_i^T @ x_i, N=256) accumulate the numerator [1, 256]; one f32 matmul
forms the denominator; reciprocal + scale; DMA out.
"""

import os
import sys

sys.path.insert(0, "/opt/trn_rl_repo")

import numpy as np

import concourse.bass as bass
import concourse.mybir as mybir
import concourse.tile as tile
from concourse import bacc
from concourse.bass import ds, ts
from concourse.bass_utils import run_bass_kernel_spmd

F32 = mybir.dt.float32
BF16 = mybir.dt.bfloat16

N_CORES = 8
B, T, D, A = 64, 4096, 256, 32
BPC = B // N_CORES          # batches per core
NCH = T // 128              # 128-row chunks per batch (32)
NG = 2                      # DMA groups per batch
RPG = NCH // NG             # chunks per group (16)
EPS = 1e-7
MASK_BIAS = 30.0            # additive pre-exp mask: s + (mask-1)*30

last_exec_time_ns = None


def _build():
    nc = bacc.Bacc(None, target_bir_lowering=False, debug=True)

    x_dram = nc.dram_tensor("x", [BPC, T, D], F32, kind="ExternalInput")
    w_dram = nc.dram_tensor("w", [128, 2 * A], F32, kind="ExternalInput")
    bbt_dram = nc.dram_tensor("bbt", [128, NCH * A], F32, kind="ExternalInput")
    ubt_dram = nc.dram_tensor("ubt", [128, NCH * A], F32, kind="ExternalInput")
    maskb_dram = nc.dram_tensor("maskb", [BPC, 128, NCH], F32, kind="ExternalInput")
    ident_dram = nc.dram_tensor("ident", [128, 128], F32, kind="ExternalInput")
    out_dram = nc.dram_tensor("out", [BPC, D], F32, kind="ExternalOutput")

    with tile.TileContext(nc) as tc:
        with (
            tc.tile_pool(name="const", bufs=1) as cpool,
            tc.tile_pool(name="xf", bufs=3) as xfpool,
            tc.tile_pool(name="xb", bufs=4) as xbpool,
            tc.tile_pool(name="xt", bufs=4) as xtpool,
            tc.tile_pool(name="ph2", bufs=2) as ph2pool,
            tc.tile_pool(name="small", bufs=2) as spool,
            tc.tile_pool(name="uitps", bufs=2, space="PSUM") as uitpool,
            tc.tile_pool(name="xtps", bufs=2, space="PSUM") as xtpspool,
            tc.tile_pool(name="ops", bufs=1, space="PSUM") as opool,
            tc.tile_pool(name="denps", bufs=1, space="PSUM") as denpool,
        ):
            # ---- constants (one-time) ----
            w_f32 = cpool.tile([128, 2 * A], F32, name="w_f32")
            nc.sync.dma_start(out=w_f32[:], in_=w_dram[:])
            w_bf = cpool.tile([128, 2 * A], BF16, name="w_bf")
            nc.vector.tensor_copy(w_bf[:], w_f32[:])

            ident = cpool.tile([128, 128], F32, name="ident")
            nc.sync.dma_start(out=ident[:], in_=ident_dram[:])
            ident_bf = cpool.tile([128, 128], BF16, name="ident_bf")
            nc.vector.tensor_copy(ident_bf[:], ident[:])

            bbt = cpool.tile([128, NCH * A], F32, name="bbt")
            nc.sync.dma_start(out=bbt[:], in_=bbt_dram[:])
            ubt = cpool.tile([128, NCH * A], F32, name="ubt")
            nc.sync.dma_start(out=ubt[:], in_=ubt_dram[:])

            ones_f = cpool.tile([128, 1], F32, name="ones_f")
            nc.vector.memset(ones_f[:], 1.0)

            for bb in range(BPC):
                uit_ps = uitpool.tile([128, NCH * A], F32, name="uit_ps", tag="uit")
                x_bf_tiles = []
                for g in range(NG):
                    x_grp = xfpool.tile([128, RPG, D], F32, name="x_grp", tag="xf")
                    nc.sync.dma_start(
                        out=x_grp[:],
                        in_=x_dram[bb][ds(2048 * g, 2048), :].rearrange(
                            "(p r) d -> p r d", r=RPG
                        ),
                    )
                    x_bf = xbpool.tile([128, RPG, D], BF16, name="x_bf", tag="xb")
                    nc.vector.tensor_copy(x_bf[:], x_grp[:])
                    x_bf_tiles.append(x_bf)

                    # transpose chunks in pairs; copy PSUM->SBUF one [128,512]
                    # tile at a time, alternating DVE/ACT
                    for rp in range(RPG // 2):
                        xt_ps = xtpspool.tile([128, 2, D], BF16, name="xt_ps", tag="xtps")
                        for rr in range(2):
                            r = 2 * rp + rr
                            for dc in range(2):
                                nc.tensor.transpose(
                                    xt_ps[:, rr, ds(128 * dc, 128)],
                                    x_bf[:, r, ds(128 * dc, 128)],
                                    ident_bf[:],
                                )
                        xt_sb = xtpool.tile([128, 2, D], BF16, name="xt_sb", tag="xt")
                        if rp % 2 == 0:
                            nc.vector.tensor_copy(xt_sb[:], xt_ps[:])
                        else:
                            nc.scalar.copy(xt_sb[:], xt_ps[:])
                        for rr in range(2):
                            i = 16 * g + 2 * rp + rr
                            nc.tensor.matmul(
                                uit_ps[:, ds(A * i, A)],
                                lhsT=xt_sb[:, rr, 0:128],
                                rhs=w_bf[:, 0:A],
                                start=True,
                                stop=False,
                            )
                            nc.tensor.matmul(
                                uit_ps[:, ds(A * i, A)],
                                lhsT=xt_sb[:, rr, 128:256],
                                rhs=w_bf[:, A : 2 * A],
                                start=False,
                                stop=True,
                            )

                # ---- phase 2: scores for the whole batch ----
                t1 = ph2pool.tile([128, NCH * A], F32, name="t1", tag="t1")
                nc.vector.tensor_add(t1[:], uit_ps[:], bbt[:])
                t2 = ph2pool.tile([128, NCH * A], F32, name="t2", tag="t2")
                nc.scalar.activation(t2[:], t1[:], mybir.ActivationFunctionType.Tanh)
                t3 = ph2pool.tile([128, NCH * A], F32, name="t3", tag="t3")
                nc.vector.tensor_mul(t3[:], t2[:], ubt[:])
                s_all = spool.tile([128, NCH, 1], F32, name="s_all", tag="s_all")
                nc.vector.reduce_sum(
                    s_all[:],
                    t3.rearrange("p (i a) -> p i a", a=A),
                    axis=mybir.AxisListType.X,
                )

                maskb = spool.tile([128, NCH], F32, name="maskb", tag="maskb")
                nc.sync.dma_start(out=maskb[:], in_=maskb_dram[bb])
                s_m = spool.tile([128, NCH], F32, name="s_m", tag="s_m")
                nc.vector.tensor_add(s_m[:], s_all[:, :, 0], maskb[:])

                e_bf = spool.tile([128, NCH], BF16, name="e_bf", tag="e_bf")
                er = spool.tile([128, 1], F32, name="er", tag="er")
                nc.scalar.activation(
                    e_bf[:],
                    s_m[:],
                    mybir.ActivationFunctionType.Exp,
                    accum_out=er[:],
                )

                den_ps = denpool.tile([1, 1], F32, name="den_ps", tag="den")
                nc.tensor.matmul(
                    den_ps[:], lhsT=er[:], rhs=ones_f[:], start=True, stop=True
                )

                # ---- phase 3: weighted sum over the sequence ----
                o_ps = opool.tile([1, D], F32, name="o_ps", tag="o")
                for i in range(NCH):
                    g, r = divmod(i, RPG)
                    nc.tensor.matmul(
                        o_ps[:],
                        lhsT=e_bf[:, ds(i, 1)],
                        rhs=x_bf_tiles[g][:, r, :],
                        start=(i == 0),
                        stop=(i == NCH - 1),
                    )

                # ---- phase 4: finalize ----
                den_sb = spool.tile([1, 1], F32, name="den_sb", tag="den_sb")
                nc.vector.tensor_scalar_add(den_sb[:], den_ps[:], EPS)
                inv = spool.tile([1, 1], F32, name="inv", tag="inv")
                nc.vector.reciprocal(inv[:], den_sb[:])
                o_sb = spool.tile([1, D], F32, name="o_sb", tag="o_sb")
                nc.vector.tensor_scalar_mul(o_sb[:], o_ps[:], inv[:])
                nc.sync.dma_start(out=out_dram[bb][None, :], in_=o_sb[:])

    nc.finalize()
    return nc


def kernel(x, mask, W, b, u):
    global last_exec_time_ns
    x = np.ascontiguousarray(np.asarray(x), dtype=np.float32)
    mask_f = np.asarray(mask).astype(np.float32)
    W = np.asarray(W, dtype=np.float32)
    b = np.asarray(b, dtype=np.float32)
    u = np.asarray(u, dtype=np.float32)

    # host-side layout prep (all tiny except x, which is only view-sliced)
    w_packed = np.ascontiguousarray(
        W.reshape(2, 128, A).transpose(1, 0, 2).reshape(128, 2 * A)
    )
    bbt = np.ascontiguousarray(np.tile(b[None, :], (128, NCH)))
    ubt = np.ascontiguousarray(np.tile(u[:, 0][None, :], (128, NCH)))
    # mask -> additive pre-exp bias, laid out [b][p][(g r)] with t = 2048g+16p+r
    maskb = np.ascontiguousarray(
        ((mask_f - 1.0) * MASK_BIAS)
        .reshape(B, NG, 128, RPG)
        .transpose(0, 2, 1, 3)
        .reshape(B, 128, NCH)
    )
    ident = np.eye(128, dtype=np.float32)

    nc = _build()

    in_maps = []
    for c in range(N_CORES):
        in_maps.append(
            {
                "x": x[c * BPC : (c + 1) * BPC],
                "w": w_packed,
                "bbt": bbt,
                "ubt": ubt,
                "maskb": maskb[c * BPC : (c + 1) * BPC],
                "ident": ident,
            }
        )

    trace = bool(int(os.environ.get("BASS_KERNEL_TRACE", "0")))
    res = run_bass_kernel_spmd(
        nc, in_maps, core_ids=list(range(N_CORES)), trace=trace
    )
    last_exec_time_ns = res.exec_time_ns

    out = np.empty((B, D), dtype=np.float32)
    for c in range(N_CORES):
        out[c * BPC : (c + 1) * BPC] = res.results[c]["out"]
    return out
